# revision 4
# baseline (speedup 1.0000x reference)
"""FISTA dictionary-learning sparse coding on 8 Trainium2 NeuronCores.

Problem: Y [8192, 784], W [784, 2048] (unit-norm columns).
  c   = power_method(W)  (largest eigenvalue of W^T W, 100 iters)
  Gamma0 = soft_threshold(Y @ (W/c), 0.1); 50 FISTA iterations; outputs
  (X = Gamma @ W.T, Gamma, norms[50]).

Strategy (data-parallel over batch, W replicated):
  - 8 cores x 1024 batch rows; each core processes 4 chunks of 256 rows.
  - All activations feature-major (transposed): Zt/Gt [2048, nb], so both
    FISTA matmuls need no on-device transposes:
      mm1: Rs = -(Z @ W.T - Y).T = (-W.T).T-matmul + identity-fold of Y
      mm2: u  = (Z - eta*(R @ W)).T = (eta*W).T-matmul + fp32 DVE add of Z
  - Matmuls in float32r (tf32): full bf16-rate on TRN2, exact products for
    tf32-rounded inputs. Z kept in fp32 for the update path (accuracy),
    with a rounded f32r copy for the PE.
  - soft_threshold(x) = relu(x-thr) - relu(-x-thr) on the scalar engine.
  - norms accumulated on-device as per-tile sum-of-squares partials,
    reduced + sqrt on host.
  - c (power method) computed on host with jax, replicating the reference
    trajectory exactly (it is not converged at 100 iters, so the value is
    X0-dependent).
"""
import sys
import numpy as np

sys.path.insert(0, "/opt/trn_rl_repo")

import concourse.bass as bass  # noqa: E402
import concourse.tile as tile  # noqa: E402
from concourse import bacc, mybir  # noqa: E402
from concourse import bass_utils  # noqa: E402

F32 = mybir.dt.float32
F32R = mybir.dt.float32r
AF = mybir.ActivationFunctionType
try:
    from concourse.alu_op_type import AluOpType
except ImportError:  # pragma: no cover
    AluOpType = mybir.AluOpType

# problem constants
B, NF, MF = 8192, 784, 2048       # batch, n (obs dim), m (code dim)
LAMBDA, FISTA_ITER, POWER_ITER = 0.1, 50, 100
N_CORES = 8
BC = B // N_CORES                 # 1024 per core
NB = 256                          # batch chunk (PE moving free dim)
NCHUNK = BC // NB                 # 4
KT = MF // 128                    # 16 tiles over m=2048
JT = 7                            # tiles over n=784
JSL = NF // JT                    # 112

_compiled = {}


def _tf32_round(x):
    i = np.ascontiguousarray(x, dtype=np.float32).view(np.uint32)
    # round-to-nearest-even on the low 13 mantissa bits
    keep = i & np.uint32(0xFFFFE000)
    rem = i & np.uint32(0x1FFF)
    half = np.uint32(0x1000)
    lsb = (i >> np.uint32(13)) & np.uint32(1)
    roundup = (rem > half) | ((rem == half) & (lsb == 1))
    out = keep + (roundup.astype(np.uint32) << np.uint32(13))
    return out.view(np.float32)


def _power_method_c(W):
    """Replicate reference.power_method exactly (jax, same PRNG + fori_loop)."""
    import jax
    import jax.numpy as jnp

    Wj = jnp.asarray(W)
    X0 = jax.random.normal(jax.random.key(1), (1, Wj.shape[1]), dtype=Wj.dtype)

    def body(i, carry):
        X, _ = carry
        X = (X @ Wj.T) @ Wj
        nm = jnp.linalg.norm(X)
        return (X / nm, nm)

    _, nm = jax.lax.fori_loop(0, POWER_ITER, body, (X0, jnp.asarray(1.0, Wj.dtype)))
    return np.float32(nm)


def _momentum_schedule():
    t = np.float32(1.0)
    moms = []
    for _ in range(FISTA_ITER):
        tn = (np.float32(1.0) + np.sqrt(np.float32(1.0) + np.float32(4.0) * t * t)) / np.float32(2.0)
        moms.append(float((t - np.float32(1.0)) / tn))
        t = tn
    return moms


def _build_program(thr, moms):
    """Build the per-core Bass program. thr = lambda/c; moms = 50 momenta."""
    nc = bacc.Bacc("TRN2", target_bir_lowering=False, debug=False,
                   num_devices=N_CORES)

    wm_d = nc.dram_tensor("wm", [MF, NF], F32R, kind="ExternalInput").ap()
    we_d = nc.dram_tensor("we", [NF, MF], F32R, kind="ExternalInput").ap()
    yt_d = nc.dram_tensor("yt", [NF, BC], F32R, kind="ExternalInput").ap()
    id_d = nc.dram_tensor("ident", [JSL, JSL], F32R, kind="ExternalInput").ap()
    xt_d = nc.dram_tensor("xt", [NF, BC], F32, kind="ExternalOutput").ap()
    gt_d = nc.dram_tensor("gt", [MF, BC], F32, kind="ExternalOutput").ap()
    nrm_d = nc.dram_tensor("nrm", [128, NCHUNK * FISTA_ITER * JT], F32,
                           kind="ExternalOutput").ap()

    with tile.TileContext(nc) as tc:
        import contextlib
        ctx = contextlib.ExitStack()
        with ctx:
            const = ctx.enter_context(tc.tile_pool(name="const", bufs=1))
            state = ctx.enter_context(tc.tile_pool(name="state", bufs=1))
            pp1 = ctx.enter_context(tc.tile_pool(name="pp1", bufs=4, space="PSUM"))
            pp2 = ctx.enter_context(tc.tile_pool(name="pp2", bufs=3, space="PSUM"))
            tmp = ctx.enter_context(tc.tile_pool(name="tmp", bufs=2))
            upool = ctx.enter_context(tc.tile_pool(name="upool", bufs=3))

            # resident weights
            wm = []
            for k in range(KT):
                t_ = const.tile([128, NF], F32R, tag=f"wm{k}")
                nc.sync.dma_start(t_[:], wm_d[k * 128:(k + 1) * 128, :])
                wm.append(t_)
            we = []
            for j in range(JT):
                t_ = const.tile([JSL, MF], F32R, tag=f"we{j}")
                nc.sync.dma_start(t_[:], we_d[j * JSL:(j + 1) * JSL, :])
                we.append(t_)
            ident = const.tile([JSL, JSL], F32R, tag="ident")
            nc.sync.dma_start(ident[:], id_d[:, :])
            bias_lam = const.tile([128, 1], F32, tag="bias_lam")
            nc.vector.memset(bias_lam[:], -LAMBDA)
            bias_thr = const.tile([128, 1], F32, tag="bias_thr")
            nc.vector.memset(bias_thr[:], float(-thr))

            # persistent per-chunk state
            zf = [state.tile([128, NB], F32, tag=f"zf{m}", name=f"zf{m}") for m in range(KT)]
            zr = [state.tile([128, NB], F32R, tag=f"zr{m}", name=f"zr{m}") for m in range(KT)]
            gt = [state.tile([128, NB], F32, tag=f"gt{m}", name=f"gtile{m}") for m in range(KT)]
            rt = [state.tile([JSL, NB], F32R, tag=f"rt{j}", name=f"rt{j}") for j in range(JT)]
            ytc = [state.tile([JSL, NB], F32R, tag=f"yt{j}", name=f"ytc{j}") for j in range(JT)]
            nrm = state.tile([128, NCHUNK * FISTA_ITER * JT], F32, tag="nrm")

            for c in range(NCHUNK):
                cs = slice(c * NB, (c + 1) * NB)
                # load Y chunk (transposed, tf32-rounded on host)
                for j in range(JT):
                    nc.sync.dma_start(ytc[j][:], yt_d[j * JSL:(j + 1) * JSL, cs])

                # ---- Gamma0 = st(eta * Y @ W, LAMBDA); Z0 = Gamma0 ----
                for m in range(KT):
                    ps = pp2.tile([128, NB], F32, tag="ps2")
                    for j in range(JT):
                        nc.tensor.matmul(ps[:], we[j][:, m * 128:(m + 1) * 128],
                                         ytc[j][:], start=(j == 0), stop=(j == JT - 1))
                    a = tmp.tile([128, NB], F32, tag="a")
                    b = tmp.tile([128, NB], F32, tag="b")
                    nc.scalar.activation(a[:], ps[:], AF.Relu, bias=bias_lam[:], scale=1.0)
                    nc.scalar.activation(b[:], ps[:], AF.Relu, bias=bias_lam[:], scale=-1.0)
                    nc.gpsimd.tensor_sub(gt[m][:], a[:], b[:])
                    nc.scalar.copy(zf[m][:], gt[m][:])
                    nc.vector.tensor_copy(zr[m][:], gt[m][:])

                # ---- 50 FISTA iterations ----
                for i in range(FISTA_ITER):
                    mom = moms[i]
                    # mm1: Rs = (-W)@Zt + I@Yt  (= -(Z@W.T - Y) transposed)
                    for j in range(JT):
                        ps = pp1.tile([JSL, NB], F32, tag="ps1")
                        nc.tensor.matmul(ps[:], ident[:], ytc[j][:],
                                         start=True, stop=False)
                        for k in range(KT):
                            nc.tensor.matmul(ps[:], wm[k][:, j * JSL:(j + 1) * JSL],
                                             zr[k][:], start=False, stop=(k == KT - 1))
                        nc.scalar.copy(rt[j][:], ps[:])
                        sq = tmp.tile([JSL, NB], F32, tag="sq")
                        col = (c * FISTA_ITER + i) * JT + j
                        nc.scalar.activation(sq[:], rt[j][:], AF.Square,
                                             accum_out=nrm[:JSL, col:col + 1])
                    # mm2 + update
                    for m in range(KT):
                        ps = pp2.tile([128, NB], F32, tag="ps2")
                        for j in range(JT):
                            nc.tensor.matmul(ps[:], we[j][:, m * 128:(m + 1) * 128],
                                             rt[j][:], start=(j == 0), stop=(j == JT - 1))
                        u = upool.tile([128, NB], F32, tag="u")
                        nc.vector.tensor_add(u[:], ps[:], zf[m][:])
                        a = tmp.tile([128, NB], F32, tag="a")
                        b = tmp.tile([128, NB], F32, tag="b")
                        nc.scalar.activation(a[:], u[:], AF.Relu, bias=bias_thr[:], scale=1.0)
                        nc.scalar.activation(b[:], u[:], AF.Relu, bias=bias_thr[:], scale=-1.0)
                        gn = tmp.tile([128, NB], F32, tag="gn")
                        nc.gpsimd.tensor_sub(gn[:], a[:], b[:])
                        d = tmp.tile([128, NB], F32, tag="d")
                        nc.vector.tensor_sub(d[:], gn[:], gt[m][:])
                        nc.vector.scalar_tensor_tensor(
                            zf[m][:], d[:], mom, gn[:],
                            op0=AluOpType.mult, op1=AluOpType.add)
                        nc.vector.tensor_copy(zr[m][:], zf[m][:])
                        nc.scalar.copy(gt[m][:], gn[:])

                # ---- outputs for this chunk ----
                for m in range(KT):
                    nc.sync.dma_start(gt_d[m * 128:(m + 1) * 128, cs], gt[m][:])
                    nc.vector.tensor_copy(zr[m][:], gt[m][:])  # Gamma in f32r
                for j in range(JT):
                    ps = pp1.tile([JSL, NB], F32, tag="ps1")
                    for k in range(KT):
                        nc.tensor.matmul(ps[:], wm[k][:, j * JSL:(j + 1) * JSL],
                                         zr[k][:], start=(k == 0), stop=(k == KT - 1))
                    xs = tmp.tile([JSL, NB], F32, tag="xs")
                    nc.scalar.mul(xs[:], ps[:], -1.0)
                    nc.sync.dma_start(xt_d[j * JSL:(j + 1) * JSL, cs], xs[:])

            nc.sync.dma_start(nrm_d[:, :], nrm[:])

    nc.compile()
    return nc


def kernel(Y, W):
    Y = np.ascontiguousarray(Y, dtype=np.float32)
    W = np.ascontiguousarray(W, dtype=np.float32)
    assert Y.shape == (B, NF) and W.shape == (NF, MF)

    c = _power_method_c(W)
    eta = np.float32(1.0) / c
    thr = float(np.float32(LAMBDA) / c)
    moms = _momentum_schedule()

    key = (float(thr), tuple(moms))
    if key not in _compiled:
        _compiled[key] = _build_program(thr, moms)
    nc = _compiled[key]

    wm_np = _tf32_round(np.float32(-1.0) * W.T)           # [2048, 784]
    we_np = _tf32_round((eta * W).astype(np.float32))     # [784, 2048]
    yt_full = _tf32_round(Y.T)                            # [784, 8192]
    id_np = np.eye(JSL, dtype=np.float32)

    in_maps = []
    for r in range(N_CORES):
        in_maps.append({
            "wm": wm_np,
            "we": we_np,
            "yt": np.ascontiguousarray(yt_full[:, r * BC:(r + 1) * BC]),
            "ident": id_np,
        })

    res = bass_utils.run_bass_kernel_spmd(nc, in_maps, core_ids=list(range(N_CORES)))
    kernel.last_results = res

    X = np.empty((B, NF), np.float32)
    Gamma = np.empty((B, MF), np.float32)
    sq = np.zeros((NCHUNK * FISTA_ITER * JT,), np.float64)
    for r, out in enumerate(res.results):
        X[r * BC:(r + 1) * BC, :] = out["xt"].T
        Gamma[r * BC:(r + 1) * BC, :] = out["gt"].T
        sq += out["nrm"].astype(np.float64).sum(axis=0)
    sq = sq.reshape(NCHUNK, FISTA_ITER, JT).sum(axis=(0, 2))   # per-iteration
    y_norm = np.linalg.norm(Y)
    norms = (np.sqrt(sq) / y_norm).astype(np.float32)
    return X, Gamma, norms
